# revision 5
# baseline (speedup 1.0000x reference)
"""EfficientAttention Trainium2 kernel.

Full-input contract: kernel(**inputs) takes the complete tensors from
setup_inputs() and returns the full output. Internally the batch dim
(n=16) is sharded across 8 NeuronCores (2 batches per core), fully
data-parallel (no collectives).

Per-core, per-batch pipeline (s = h*w = 4096, ch = 256, 8 heads x 32):
  phase 1: kT/vT projections in [s, ch] layout (bf16 Xf s-chunks are the
           matmul stationary operand so no transpose is ever needed),
           exp(kT) on ScalarE during PSUM eviction (bk cancels in the
           softmax over s), vT evicted with an appended ones column so
           the ctx matmul also produces the softmax-over-s normalizer
           S[k] as its last output column.  ctx accumulates over all 32
           s-chunks in PSUM (bf16 inputs, fp32 accumulation).
  phase 2: ctx epilogue: ctx = blockmask * (ctx_raw / S) + mask*bv
           (bv enters ctx exactly because softmax rows sum to 1).
  phase 3 (per 512-wide s-chunk): q projection ([ch, s] layout),
           eq = exp(q + bq) on ScalarE; per-head softmax-over-ck
           denominator D broadcast to all 128 partitions with one
           block-diagonal ones matmul; recip via fast DVE approx;
           att = (ctx^T @ eq) * recipD fused into the PSUM eviction;
           output projection; (+br +X) fused into one
           scalar_tensor_tensor eviction; DMA out.

All matmuls run bf16 (full PE rate, fp32 PSUM accumulation); the fp32 X
residual is streamed from DRAM per-chunk for the epilogue only.
"""

import sys

if "/opt/trn_rl_repo" not in sys.path:
    sys.path.insert(0, "/opt/trn_rl_repo")

from contextlib import ExitStack

import ml_dtypes
import numpy as np

import concourse.bass as bass
import concourse.mybir as mybir
import concourse.tile as tile
from concourse import bacc
from concourse.alu_op_type import AluOpType
from concourse.bass_utils import run_bass_kernel_spmd

N_CORES = 8
B_PER_CORE = 2
CH = 256          # in/key/value channels
S = 4096          # spatial positions (64*64)
N_HEADS = 8
HD = 32           # channels per head
NT = 2            # 128-partition tiles over the 256 channels
SC1 = 128         # phase-1 s-chunk (stationary free dim)
NSC1 = S // SC1   # 32
SC3 = 512         # phase-3 s-chunk (moving free dim)
NSC3 = S // SC3   # 8

F32 = mybir.dt.float32
BF16 = mybir.dt.bfloat16
EXP = mybir.ActivationFunctionType.Exp
MUL = AluOpType.mult
ADD = AluOpType.add
BF = ml_dtypes.bfloat16


def build_bass():
    nc = bacc.Bacc(
        "TRN2",
        target_bir_lowering=False,
        debug=False,
        enable_asserts=False,
        num_devices=N_CORES,
    )

    X = nc.dram_tensor("X", [B_PER_CORE, CH, S], F32, kind="ExternalInput").ap()
    XB = nc.dram_tensor("XB", [B_PER_CORE, CH, S], BF16, kind="ExternalInput").ap()
    WKT = nc.dram_tensor("WKT", [128, NT, CH], BF16, kind="ExternalInput").ap()
    WVT = nc.dram_tensor("WVT", [128, NT, CH], BF16, kind="ExternalInput").ap()
    WQT = nc.dram_tensor("WQT", [128, NT, NT, 128], BF16, kind="ExternalInput").ap()
    WRT = nc.dram_tensor("WRT", [128, NT, NT, 128], BF16, kind="ExternalInput").ap()
    BQ = nc.dram_tensor("BQ", [128, NT], F32, kind="ExternalInput").ap()
    BR = nc.dram_tensor("BR", [128, NT], F32, kind="ExternalInput").ap()
    MASKB = nc.dram_tensor("MASKB", [128, 128], BF16, kind="ExternalInput").ap()
    CMASK = nc.dram_tensor("CMASK", [NT, 128, CH], F32, kind="ExternalInput").ap()
    BVMASK = nc.dram_tensor("BVMASK", [NT, 128, CH], F32, kind="ExternalInput").ap()
    Y = nc.dram_tensor("Y", [B_PER_CORE, CH, S], F32, kind="ExternalOutput").ap()

    with ExitStack() as ctx:
        tc = ctx.enter_context(tile.TileContext(nc))
        singles = ctx.enter_context(tc.tile_pool(name="singles", bufs=1))
        big = ctx.enter_context(tc.tile_pool(name="big", bufs=2))
        chunks = ctx.enter_context(tc.tile_pool(name="chunks", bufs=3))

        wkt = singles.tile([128, NT, CH], BF16)
        nc.sync.dma_start(out=wkt, in_=WKT)
        wvt = singles.tile([128, NT, CH], BF16)
        nc.sync.dma_start(out=wvt, in_=WVT)
        wqt = singles.tile([128, NT, NT, 128], BF16)
        nc.sync.dma_start(out=wqt, in_=WQT)
        wrt = singles.tile([128, NT, NT, 128], BF16)
        nc.sync.dma_start(out=wrt, in_=WRT)
        bq_sb = singles.tile([128, NT], F32)
        nc.sync.dma_start(out=bq_sb, in_=BQ)
        br_sb = singles.tile([128, NT], F32)
        nc.sync.dma_start(out=br_sb, in_=BR)
        maskb_sb = singles.tile([128, 128], BF16)
        nc.sync.dma_start(out=maskb_sb, in_=MASKB)
        cmask_sb = singles.tile([128, NT, CH], F32)
        bvmask_sb = singles.tile([128, NT, CH], F32)
        for kb in range(NT):
            nc.sync.dma_start(out=cmask_sb[:, kb, :], in_=CMASK[kb])
            nc.sync.dma_start(out=bvmask_sb[:, kb, :], in_=BVMASK[kb])

        for b in range(B_PER_CORE):
            # ---- input load (bf16 copy for matmuls) -------------------------
            xb = big.tile([128, NT, S], BF16, tag="xb")
            for ct in range(NT):
                nc.sync.dma_start(out=xb[:, ct, :], in_=XB[b, ct * 128:(ct + 1) * 128, :])

            ekt = big.tile([128, NSC1, CH], BF16, tag="ekt")
            vt = big.tile([128, NSC1, CH + 1], BF16, tag="vt")
            nc.vector.memset(vt[:, :, CH], 1.0)

            with tc.tile_pool(name=f"psA{b}", bufs=2, space="PSUM") as psA:
                ctxps = psA.tile(
                    [128, NT, CH + 1], F32, tag="ctxps", bufs=1,
                    padded_shape=[128, NT, 512],
                )
                # ---- phase 1: kT, vT, ctx accumulation ----------------------
                for c in range(NSC1):
                    kps = psA.tile([128, CH], F32, tag="kps")
                    vps = psA.tile([128, CH], F32, tag="vps")
                    for ct in range(NT):
                        xsl = xb[:, ct, c * SC1:(c + 1) * SC1]
                        nc.tensor.matmul(
                            kps, lhsT=xsl, rhs=wkt[:, ct, :],
                            start=(ct == 0), stop=(ct == NT - 1),
                        )
                    for ct in range(NT):
                        xsl = xb[:, ct, c * SC1:(c + 1) * SC1]
                        nc.tensor.matmul(
                            vps, lhsT=xsl, rhs=wvt[:, ct, :],
                            start=(ct == 0), stop=(ct == NT - 1),
                        )
                    nc.scalar.activation(out=ekt[:, c, :], in_=kps, func=EXP)
                    nc.scalar.copy(out=vt[:, c, 0:CH], in_=vps)
                    for kb in range(NT):
                        nc.tensor.matmul(
                            ctxps[:, kb, :],
                            lhsT=ekt[:, c, kb * 128:(kb + 1) * 128],
                            rhs=vt[:, c, :],
                            start=(c == 0), stop=(c == NSC1 - 1),
                            skip_group_check=True,
                        )

                # ---- phase 2: ctx epilogue ----------------------------------
                ctxm = chunks.tile([128, NT, CH], BF16, tag="ctxm", bufs=2)
                rs = chunks.tile([128, NT, 1], F32, tag="rs", bufs=2)
                for kb in range(NT):
                    nc.vector.reciprocal(rs[:, kb, :], ctxps[:, kb, CH:CH + 1])
                    nc.vector.scalar_tensor_tensor(
                        out=ctxm[:, kb, :], in0=ctxps[:, kb, 0:CH],
                        scalar=rs[:, kb, :], in1=cmask_sb[:, kb, :],
                        op0=MUL, op1=MUL,
                    )
                    nc.vector.tensor_tensor(
                        out=ctxm[:, kb, :], in0=ctxm[:, kb, :],
                        in1=bvmask_sb[:, kb, :], op=ADD,
                    )

            # ---- phase 3: q, softmax-over-ck, att, out projection ----------
            with tc.tile_pool(name=f"psB{b}", bufs=2, space="PSUM") as psB:
                for j in range(NSC3):
                    jsl = slice(j * SC3, (j + 1) * SC3)
                    xepi = chunks.tile([128, NT, SC3], F32, tag="xepi")
                    for ot in range(NT):
                        nc.sync.dma_start(
                            out=xepi[:, ot, :],
                            in_=X[b, ot * 128:(ot + 1) * 128, jsl],
                        )
                    eq = chunks.tile([128, NT, SC3], BF16, tag="eq")
                    recb = chunks.tile([128, NT, SC3], F32, tag="recb")
                    for ot in range(NT):
                        qps = psB.tile([128, SC3], F32, tag="qps")
                        for ct in range(NT):
                            nc.tensor.matmul(
                                qps, lhsT=wqt[:, ct, ot, :],
                                rhs=xb[:, ct, jsl],
                                start=(ct == 0), stop=(ct == NT - 1),
                            )
                        nc.scalar.activation(
                            out=eq[:, ot, :], in_=qps, func=EXP,
                            bias=bq_sb[:, ot:ot + 1], scale=1.0,
                        )
                        dps = psB.tile([128, SC3], F32, tag="dps")
                        nc.tensor.matmul(
                            dps, lhsT=maskb_sb, rhs=eq[:, ot, :],
                            start=True, stop=True,
                        )
                        nc.vector.reciprocal_approx_fast(
                            out=recb[:, ot, :], in_=dps,
                        )
                    att0 = chunks.tile([128, SC3], BF16, tag="att0")
                    att1 = chunks.tile([128, SC3], BF16, tag="att1")
                    for vt_i, att_sb in ((0, att0), (1, att1)):
                        aps = psB.tile([128, SC3], F32, tag="aps")
                        for kt in range(NT):
                            nc.tensor.matmul(
                                aps,
                                lhsT=ctxm[:, kt, vt_i * 128:(vt_i + 1) * 128],
                                rhs=eq[:, kt, :],
                                start=(kt == 0), stop=(kt == NT - 1),
                            )
                        nc.vector.scalar_tensor_tensor(
                            out=att_sb, in0=aps, scalar=1.0,
                            in1=recb[:, vt_i, :], op0=MUL, op1=MUL,
                        )
                    for ot in range(NT):
                        ops = psB.tile([128, SC3], F32, tag="ops")
                        for vt_i, att_sb in ((0, att0), (1, att1)):
                            nc.tensor.matmul(
                                ops, lhsT=wrt[:, vt_i, ot, :],
                                rhs=att_sb,
                                start=(vt_i == 0), stop=(vt_i == NT - 1),
                            )
                        osb = chunks.tile([128, SC3], F32, tag="osb")
                        nc.vector.scalar_tensor_tensor(
                            out=osb, in0=ops, scalar=br_sb[:, ot:ot + 1],
                            in1=xepi[:, ot, :], op0=ADD, op1=ADD,
                        )
                        nc.sync.dma_start(
                            out=Y[b, ot * 128:(ot + 1) * 128, jsl], in_=osb,
                        )
    nc.compile()
    return nc


def make_shared_inputs(Wk, bk, Wq, bq, Wv, bv, Wr, br):
    f = np.float32
    del bk  # bk cancels in softmax over s

    def ct_tiles(W):
        # [c, o] -> [p, ct, o]
        return np.ascontiguousarray(
            np.asarray(W, f).T.reshape(NT, 128, CH).transpose(1, 0, 2)
        ).astype(BF)

    def ct_ot_tiles(W):
        # [c, o] -> [p, ct, ot, m]
        return np.ascontiguousarray(
            np.asarray(W, f).T.reshape(NT, 128, NT, 128).transpose(1, 0, 2, 3)
        ).astype(BF)

    p = np.arange(128)
    m = np.arange(128)
    maskb = (p[:, None] // HD == m[None, :] // HD).astype(BF)

    v = np.arange(CH)
    cmask = np.zeros((NT, 128, CH), f)
    for kt in range(NT):
        heads = (kt * 128 + p) // HD
        cmask[kt] = (heads[:, None] == v[None, :] // HD).astype(f)
    bvmask = cmask * np.asarray(bv, f)[None, None, :]

    return {
        "WKT": ct_tiles(Wk),
        "WVT": ct_tiles(Wv),
        "WQT": ct_ot_tiles(Wq),
        "WRT": ct_ot_tiles(Wr),
        "BQ": np.ascontiguousarray(np.asarray(bq, f).reshape(NT, 128).T),
        "BR": np.ascontiguousarray(np.asarray(br, f).reshape(NT, 128).T),
        "MASKB": maskb,
        "CMASK": cmask,
        "BVMASK": bvmask,
    }


_CACHE = {}


def _get_nc():
    if "nc" not in _CACHE:
        _CACHE["nc"] = build_bass()
    return _CACHE["nc"]


def kernel(X, Wk, bk, Wq, bq, Wv, bv, Wr, br, trace=False):
    X = np.asarray(X, np.float32)
    n = X.shape[0]
    assert n == N_CORES * B_PER_CORE, X.shape
    shared = make_shared_inputs(Wk, bk, Wq, bq, Wv, bv, Wr, br)
    nc = _get_nc()
    in_maps = []
    for i in range(N_CORES):
        xs = np.ascontiguousarray(
            X[i * B_PER_CORE:(i + 1) * B_PER_CORE].reshape(B_PER_CORE, CH, S)
        )
        m = dict(shared)
        m["X"] = xs
        m["XB"] = xs.astype(BF)
        in_maps.append(m)
    res = run_bass_kernel_spmd(
        nc, in_maps, core_ids=list(range(N_CORES)), trace=trace,
    )
    out = np.concatenate([r["Y"] for r in res.results], axis=0)
    out = out.reshape(n, CH, 64, 64)
    if trace:
        return out, res
    return out
